# revision 1
# baseline (speedup 1.0000x reference)
"""Trainium2 Bass kernel for nn_Classifier (GNN edge-MLP link predictor).

Computes, for E candidate edges:
    out[e] = W2 . relu( x_nc[i0[e]] @ W1[:H] + x_pr[i1[e]] @ W1[H:] + b1 ) + b2

Strategy (8 NeuronCores, data-parallel over edges):
  Phase 1 (per core): precompute per-node partial activations
      a[n] = x_nc[n] @ W1[:H] + b1/2,   b[n] = x_pr[n] @ W1[H:] + b1/2
  as fp16 [N, 128] row-major tables in DRAM scratch (tensor engine,
  stationary = transposed x chunks, moving = W1 half).

  Phase 2: edges sharded 125k/core. Per tile of T edges, two
  NON-transpose dma_gathers (edge-major layout [128 edges, 128 feat])
  pull a[i0] and b[i1] rows. Descriptor generation is the bottleneck
  engine (GpSimd/Pool SWDGE), so gathers are spread round-robin over 4
  SWDGE queues, which the Q7 cluster executes on distinct CPU pairs
  concurrently (~4x). Transpose-mode gathers cannot do this (shared
  xbar state corrupts concurrent streams); edge-major avoids the xbar,
  and the MLP becomes elementwise + a free-axis reduction:
      h = relu(gA + gB); out = reduce_f(h * w2) + b2
  on DVE (3 passes/tile) + scalar engine (bias), no per-edge matmul.

Output lands as [128, G] per tile (edge g*128+p at partition p, col g);
the host transposes back. fp16 tables + fp16 elementwise, fp32 reduce:
rel err ~1e-3 vs fp32 reference.
"""

import numpy as np
import ml_dtypes

import concourse.bass as bass
import concourse.tile as tile
from concourse import bacc, mybir
from concourse import bass_utils

F32 = mybir.dt.float32
FP16 = mybir.dt.float16
BF16 = mybir.dt.bfloat16
I16 = mybir.dt.int16

N_CORES = 8
H = 128

# Full-problem geometry (hardcoded per the task contract).
E_TOTAL = 1_000_000
N_NODES = 20_000
NP = 20_480  # node tables padded to 40 chunks of 512

T_MAIN = 8192


def _tiles_for(e_core: int, t_main: int):
    """Tile sizes (multiples of 128) covering e_core with minimal padding."""
    n_full = e_core // t_main
    rem = e_core - n_full * t_main
    tiles = [t_main] * n_full
    if rem:
        tiles.append(((rem + 127) // 128) * 128)
    return tiles


def _build(tiles: list, reps: int = 1):
    e_pad = sum(tiles)
    assert all(t % 128 == 0 and t <= T_MAIN for t in tiles)
    g_tot = e_pad // 128

    nc = bacc.Bacc(
        "TRN2",
        target_bir_lowering=False,
        debug=False,
        num_devices=N_CORES,
        num_swdge_queues=4,
    )

    xt_nc = nc.dram_tensor("xt_nc", [H, NP], BF16, kind="ExternalInput").ap()
    xt_pr = nc.dram_tensor("xt_pr", [H, NP], BF16, kind="ExternalInput").ap()
    idx0 = nc.dram_tensor("idx0", [16, e_pad // 16], I16, kind="ExternalInput").ap()
    idx1 = nc.dram_tensor("idx1", [16, e_pad // 16], I16, kind="ExternalInput").ap()
    w1nc = nc.dram_tensor("w1nc", [H, H], BF16, kind="ExternalInput").ap()
    w1pr = nc.dram_tensor("w1pr", [H, H], BF16, kind="ExternalInput").ap()
    hb1r = nc.dram_tensor("hb1r", [1, H], BF16, kind="ExternalInput").ap()
    w2r = nc.dram_tensor("w2r", [1, T_MAIN], FP16, kind="ExternalInput").ap()
    b2 = nc.dram_tensor("b2", [1, 1], F32, kind="ExternalInput").ap()
    out = nc.dram_tensor("out", [128, g_tot], F32, kind="ExternalOutput").ap()

    a_tbl = nc.dram_tensor("a_tbl", [NP, H], FP16, kind="Internal").ap()
    b_tbl = nc.dram_tensor("b_tbl", [NP, H], FP16, kind="Internal").ap()

    relu_max = mybir.AluOpType.max
    add_op = mybir.AluOpType.add
    mult_op = mybir.AluOpType.mult
    bypass = mybir.AluOpType.bypass
    ident = mybir.ActivationFunctionType.Identity

    with tile.TileContext(nc) as tc:
        with (
            tc.tile_pool(name="const", bufs=1) as cpool,
            tc.tile_pool(name="idx", bufs=1) as ipool,
            tc.tile_pool(name="x", bufs=3) as xpool,
            tc.tile_pool(name="ao", bufs=3) as apool,
            tc.tile_pool(name="gather", bufs=3) as gpool,
            tc.tile_pool(name="h", bufs=1) as hpool,
            tc.tile_pool(name="stage", bufs=2) as spool,
            tc.tile_pool(name="ps", bufs=4, space="PSUM") as pspool,
        ):
            # ---- constants ----
            w1nc_sb = cpool.tile([H, H], BF16, tag="w1nc")
            nc.sync.dma_start(w1nc_sb[:], w1nc[:])
            w1pr_sb = cpool.tile([H, H], BF16, tag="w1pr")
            nc.sync.dma_start(w1pr_sb[:], w1pr[:])
            hb1_row = cpool.tile([1, H], BF16, tag="hb1row")
            nc.sync.dma_start(hb1_row[:], hb1r[:])
            ones_sb = cpool.tile([1, H], BF16, tag="ones")
            nc.vector.memset(ones_sb[:], 1.0)
            w2_row = cpool.tile([1, T_MAIN], FP16, tag="w2row")
            nc.sync.dma_start(w2_row[:], w2r[:])
            b2_sb = cpool.tile([1, 1], F32, tag="b2")
            nc.sync.dma_start(b2_sb[:], b2[:])

            w2_rep = cpool.tile([128, T_MAIN], FP16, tag="w2rep")
            nc.gpsimd.partition_broadcast(w2_rep[:], w2_row[:])
            b2_rep = cpool.tile([128, 1], F32, tag="b2rep")
            nc.gpsimd.partition_broadcast(b2_rep[:], b2_sb[:])

            # ---- indices: [16, N] wrapped, replicated across partition groups
            idx0_sb = ipool.tile([128, e_pad // 16], I16, tag="idx0")
            idx1_sb = ipool.tile([128, e_pad // 16], I16, tag="idx1")
            for k in range(8):
                nc.sync.dma_start(idx0_sb[16 * k : 16 * (k + 1), :], idx0[:])
                nc.sync.dma_start(idx1_sb[16 * k : 16 * (k + 1), :], idx1[:])

            # ---- phase 1: a/b node tables ----
            for tbl, w1_sb, xt in (
                (a_tbl, w1nc_sb, xt_nc),
                (b_tbl, w1pr_sb, xt_pr),
            ):
                tbl_v = tbl.rearrange("(b p) f -> p b f", p=128)
                for c in range(NP // 512):
                    sl = slice(c * 512, (c + 1) * 512)
                    xc = xpool.tile([H, 512], BF16, tag="xc")
                    nc.sync.dma_start(xc[:], xt[:, sl])
                    ps = pspool.tile([128, 512], F32, tag="ps")
                    for k in range(4):
                        ks = slice(k * 128, (k + 1) * 128)
                        nc.tensor.matmul(
                            ps[:, ks], ones_sb[:], hb1_row[:], start=True, stop=False
                        )
                        nc.tensor.matmul(
                            ps[:, ks], xc[:, ks], w1_sb[:], start=False, stop=True
                        )
                    ao = apool.tile([128, 512], FP16, tag="ao")
                    nc.scalar.activation(ao[:], ps[:], ident)
                    nc.sync.dma_start(
                        tbl_v[:, 4 * c : 4 * c + 4, :],
                        ao[:].rearrange("p (k f) -> p k f", k=4),
                    )

            # ---- phase 2: edge loop ----
            seq = [t for _ in range(reps) for t in enumerate(tiles)]
            for i, (ti, t) in enumerate(seq):
                g = t // 128
                g0 = sum(tiles[:ti]) // 128
                c0 = sum(tiles[:ti]) // 16
                ic = t // 16

                gA = gpool.tile([128, T_MAIN], FP16, tag="gA")
                nc.gpsimd.dma_gather(
                    gA[:, :t].rearrange("p (g f) -> p g f", f=H),
                    a_tbl,
                    idx0_sb[:, c0 : c0 + ic],
                    t,
                    t,
                    H,
                    transpose=False,
                    single_packet=False,
                    queue_num=(2 * i) % 4,
                )
                gB = gpool.tile([128, T_MAIN], FP16, tag="gB")
                nc.gpsimd.dma_gather(
                    gB[:, :t].rearrange("p (g f) -> p g f", f=H),
                    b_tbl,
                    idx1_sb[:, c0 : c0 + ic],
                    t,
                    t,
                    H,
                    transpose=False,
                    single_packet=False,
                    queue_num=(2 * i + 1) % 4,
                )

                h = hpool.tile([128, T_MAIN], FP16, tag="h")
                nc.vector.tensor_tensor(h[:, :t], gA[:, :t], gB[:, :t], add_op)
                m = hpool.tile([128, T_MAIN], FP16, tag="m")
                nc.vector.scalar_tensor_tensor(
                    m[:, :t], h[:, :t], 0.0, w2_rep[:, :t], relu_max, mult_op
                )
                red = spool.tile([128, T_MAIN // 128], F32, tag="red")
                nc.vector.tensor_reduce(
                    red[:, :g],
                    m[:, :t].rearrange("p (g f) -> p g f", f=H),
                    mybir.AxisListType.X,
                    add_op,
                )
                stage = spool.tile([128, T_MAIN // 128], F32, tag="stage")
                nc.scalar.activation(stage[:, :g], red[:, :g], ident, bias=b2_rep[:])
                nc.sync.dma_start(out[:, g0 : g0 + g], stage[:, :g])

    nc.compile()
    return nc


# ---------------------------------------------------------------------------
# Host-side wrapper
# ---------------------------------------------------------------------------

_CACHE: dict = {}


def _wrap_idx(idx: np.ndarray, e_pad: int) -> np.ndarray:
    """int16 [16, e_pad//16] with index i at [i % 16, i // 16]."""
    pad = np.zeros(e_pad, np.int16)
    pad[: idx.shape[0]] = idx.astype(np.int16)
    return np.ascontiguousarray(pad.reshape(e_pad // 16, 16).T)


def _get_program(tiles):
    key = tuple(tiles)
    if key not in _CACHE:
        _CACHE[key] = _build(list(tiles))
    return _CACHE[key]


def kernel(
    x_ncRNA: np.ndarray,
    x_Protein: np.ndarray,
    edge_label_index: np.ndarray,
    W1: np.ndarray,
    b1: np.ndarray,
    W2: np.ndarray,
    b2: np.ndarray,
    _trace: bool = False,
) -> np.ndarray:
    E = edge_label_index.shape[1]
    n_nodes = x_ncRNA.shape[0]
    assert E % N_CORES == 0 and n_nodes <= NP
    e_core = E // N_CORES
    tiles = _tiles_for(e_core, T_MAIN)
    e_pad = sum(tiles)
    g_tot = e_pad // 128

    nc = _get_program(tiles)

    def prep_xt(x):
        xt = np.zeros((H, NP), ml_dtypes.bfloat16)
        xt[:, :n_nodes] = x.T.astype(ml_dtypes.bfloat16)
        return np.ascontiguousarray(xt)

    xt_nc = prep_xt(x_ncRNA)
    xt_pr = prep_xt(x_Protein)
    w1nc = np.ascontiguousarray(W1[:H].astype(ml_dtypes.bfloat16))
    w1pr = np.ascontiguousarray(W1[H:].astype(ml_dtypes.bfloat16))
    hb1r = np.ascontiguousarray(
        (0.5 * b1).astype(ml_dtypes.bfloat16).reshape(1, H)
    )
    w2r = np.ascontiguousarray(
        np.tile(W2[:, 0].astype(np.float16), T_MAIN // H).reshape(1, T_MAIN)
    )
    b2_ = np.ascontiguousarray(b2.reshape(1, 1).astype(np.float32))

    in_maps = []
    for c in range(N_CORES):
        sl = slice(c * e_core, (c + 1) * e_core)
        in_maps.append(
            {
                "xt_nc": xt_nc,
                "xt_pr": xt_pr,
                "idx0": _wrap_idx(np.asarray(edge_label_index[0, sl]), e_pad),
                "idx1": _wrap_idx(np.asarray(edge_label_index[1, sl]), e_pad),
                "w1nc": w1nc,
                "w1pr": w1pr,
                "hb1r": hb1r,
                "w2r": w2r,
                "b2": b2_,
            }
        )

    res = bass_utils.run_bass_kernel_spmd(
        nc, in_maps, core_ids=list(range(N_CORES)), trace=_trace
    )
    out = np.empty(E, np.float32)
    for c in range(N_CORES):
        # out[p, g] = edge g*128 + p of this core
        flat = res.results[c]["out"].T.reshape(-1)
        out[c * e_core : (c + 1) * e_core] = flat[:e_core]
    kernel._last_results = res
    return out



# revision 2
# speedup vs baseline: 1.1604x; 1.1604x over previous
"""Trainium2 Bass kernel for nn_Classifier (GNN edge-MLP link predictor).

Computes, for E candidate edges:
    out[e] = W2 . relu( x_nc[i0[e]] @ W1[:H] + x_pr[i1[e]] @ W1[H:] + b1 ) + b2

v2 strategy ("swept-A + gathered-B", 8 cores):
  The v1 bottleneck was SWDGE descriptor generation on the GpSimd Q7
  cluster: ~2.6 ns/gathered-row aggregate across all 4 queues (hard
  saturation), i.e. ~640us for 250k rows/core.  v2 removes the a-side
  descriptors entirely:

  * Edges are range-sharded by i0: core k owns the ~125k edges whose
    i0 falls in its contiguous ~2500-row slice of the ncRNA table
    (multiplicity ~50 edges/node within the core).
  * Side A (i0): "sweep layers".  Host assigns each edge an occurrence
    rank (occ) within its i0 value; edges with occ < K=46 go to slot
    (p = v%128, col = occ*20 + v//128).  Layer occ is then filled by a
    single plain HWDGE strided DMA of the whole a-slice - zero
    descriptors-per-edge.  Leftover duplicates (occ >= K, ~13k) use one
    small dma_gather.  Empty slots waste a b-descriptor + epilogue work
    but no a-work.
  * Side B (i1): per-tile dma_gather from the full b-table into the
    edge slots dictated by side A (~131k descriptors vs 250k in v1).
  * Epilogue per tile [128 x 8192 slots]: DVE add (fp16 2x rate),
    Scalar relu (offloaded from DVE), DVE mult by replicated w2, DVE
    free-axis reduce, Scalar +b2, DMA out.  Keeps DVE (~17us/tile)
    under the Q7 gather time.
  * Phase 1 builds b_tbl with pure matmuls (bias seeds only on the
    5-chunk a-slice), so b-gathers unblock at ~50us.

Output lands as [128, COLS] f32 slot grid per core; the host scatters
slots back to original edge positions.
"""

import numpy as np
import ml_dtypes

import concourse.bass as bass
import concourse.tile as tile
from concourse import bacc, mybir
from concourse import bass_utils

F32 = mybir.dt.float32
FP16 = mybir.dt.float16
BF16 = mybir.dt.bfloat16
I16 = mybir.dt.int16

N_CORES = 8
H = 128

E_TOTAL = 1_000_000
N_NODES = 20_000
NP = 20_480         # b table rows (full, padded to 40 chunks of 512)
WIDTH = 2_560       # a table slice rows (padded; 20 cols of 128, 5 chunks of 512)
K_LAYERS = 46       # sweep layers
LCOLS = WIDTH // 128            # 20 cols per layer
SWEEP_COLS = K_LAYERS * LCOLS   # 920
SINGLE_COLS = 144               # singles capacity 18432
TILE_COLS = 64
T = TILE_COLS * 128             # 8192 slots per tile
COLS = 1088                     # 17 tiles
N_TILES = COLS // TILE_COLS
SINGLE_BASE = SWEEP_COLS
E_CORE = E_TOTAL // N_CORES

add_op = mybir.AluOpType.add
mult_op = mybir.AluOpType.mult
ident = mybir.ActivationFunctionType.Identity
relu_fn = mybir.ActivationFunctionType.Relu


def _build():
    nc = bacc.Bacc(
        "TRN2",
        target_bir_lowering=False,
        debug=False,
        num_devices=N_CORES,
        num_swdge_queues=4,
    )

    xt_pr = nc.dram_tensor("xt_pr", [H, NP], BF16, kind="ExternalInput").ap()
    xt_nc = nc.dram_tensor("xt_nc", [H, WIDTH], BF16, kind="ExternalInput").ap()
    w1nc = nc.dram_tensor("w1nc", [H, H], BF16, kind="ExternalInput").ap()
    w1pr = nc.dram_tensor("w1pr", [H, H], BF16, kind="ExternalInput").ap()
    b1r = nc.dram_tensor("b1r", [1, H], BF16, kind="ExternalInput").ap()
    w2r = nc.dram_tensor("w2r", [1, T], FP16, kind="ExternalInput").ap()
    b2 = nc.dram_tensor("b2", [1, 1], F32, kind="ExternalInput").ap()
    idxB = nc.dram_tensor("idxB", [16, COLS * 8], I16, kind="ExternalInput").ap()
    idxA = nc.dram_tensor("idxA", [16, SINGLE_COLS * 8], I16, kind="ExternalInput").ap()
    out = nc.dram_tensor("out", [128, COLS], F32, kind="ExternalOutput").ap()

    a_tbl = nc.dram_tensor("a_tbl", [WIDTH, H], FP16, kind="Internal").ap()
    b_tbl = nc.dram_tensor("b_tbl", [NP, H], FP16, kind="Internal").ap()

    with tile.TileContext(nc) as tc:
        with (
            tc.tile_pool(name="const", bufs=1) as cpool,
            tc.tile_pool(name="idx", bufs=1) as ipool,
            tc.tile_pool(name="x", bufs=3) as xpool,
            tc.tile_pool(name="ao", bufs=3) as apool,
            tc.tile_pool(name="gA", bufs=3) as gApool,
            tc.tile_pool(name="gB", bufs=3) as gBpool,
            tc.tile_pool(name="h", bufs=2) as hpool,
            tc.tile_pool(name="stage", bufs=2) as spool,
            tc.tile_pool(name="ps", bufs=4, space="PSUM") as pspool,
        ):
            # ---- constants ----
            w1nc_sb = cpool.tile([H, H], BF16, tag="w1nc")
            nc.sync.dma_start(w1nc_sb[:], w1nc[:])
            w1pr_sb = cpool.tile([H, H], BF16, tag="w1pr")
            nc.sync.dma_start(w1pr_sb[:], w1pr[:])
            b1_row = cpool.tile([1, H], BF16, tag="b1row")
            nc.sync.dma_start(b1_row[:], b1r[:])
            ones_sb = cpool.tile([1, H], BF16, tag="ones")
            nc.vector.memset(ones_sb[:], 1.0)
            w2_row = cpool.tile([1, T], FP16, tag="w2row")
            nc.sync.dma_start(w2_row[:], w2r[:])
            b2_sb = cpool.tile([1, 1], F32, tag="b2")
            nc.sync.dma_start(b2_sb[:], b2[:])

            w2_rep = cpool.tile([128, T], FP16, tag="w2rep")
            nc.gpsimd.partition_broadcast(w2_rep[:], w2_row[:])
            b2_rep = cpool.tile([128, 1], F32, tag="b2rep")
            nc.gpsimd.partition_broadcast(b2_rep[:], b2_sb[:])

            # ---- indices (wrapped by 16, replicated to 8 groups) ----
            idxB_sb = ipool.tile([128, COLS * 8], I16, tag="idxB")
            idxA_sb = ipool.tile([128, SINGLE_COLS * 8], I16, tag="idxA")
            for k in range(8):
                nc.sync.dma_start(idxB_sb[16 * k : 16 * (k + 1), :], idxB[:])
                nc.sync.dma_start(idxA_sb[16 * k : 16 * (k + 1), :], idxA[:])

            # ---- phase 1: b table (full, no bias), then a slice (with bias) ----
            b_view = b_tbl.rearrange("(b p) f -> p b f", p=128)
            for c in range(NP // 512):
                sl = slice(c * 512, (c + 1) * 512)
                xc = xpool.tile([H, 512], BF16, tag="xc")
                nc.sync.dma_start(xc[:], xt_pr[:, sl])
                ps = pspool.tile([128, 512], F32, tag="ps")
                for k in range(4):
                    ks = slice(k * 128, (k + 1) * 128)
                    nc.tensor.matmul(
                        ps[:, ks], xc[:, ks], w1pr_sb[:], start=True, stop=True
                    )
                ao = apool.tile([128, 512], FP16, tag="ao")
                nc.scalar.activation(ao[:], ps[:], ident)
                nc.sync.dma_start(
                    b_view[:, 4 * c : 4 * c + 4, :],
                    ao[:].rearrange("p (k f) -> p k f", k=4),
                )
            a_view = a_tbl.rearrange("(b p) f -> p b f", p=128)
            for c in range(WIDTH // 512):
                sl = slice(c * 512, (c + 1) * 512)
                xc = xpool.tile([H, 512], BF16, tag="xc")
                nc.sync.dma_start(xc[:], xt_nc[:, sl])
                ps = pspool.tile([128, 512], F32, tag="ps")
                for k in range(4):
                    ks = slice(k * 128, (k + 1) * 128)
                    nc.tensor.matmul(
                        ps[:, ks], ones_sb[:], b1_row[:], start=True, stop=False
                    )
                    nc.tensor.matmul(
                        ps[:, ks], xc[:, ks], w1nc_sb[:], start=False, stop=True
                    )
                ao = apool.tile([128, 512], FP16, tag="ao")
                nc.scalar.activation(ao[:], ps[:], ident)
                nc.sync.dma_start(
                    a_view[:, 4 * c : 4 * c + 4, :],
                    ao[:].rearrange("p (k f) -> p k f", k=4),
                )

            # layer view of the a slice: slot (p, c) <-> row c*128 + p
            a_slot = a_tbl.rearrange("(c p) f -> p c f", p=128)

            # ---- phase 2: per-tile sweep / gather / MLP ----
            qn = 0
            for t in range(N_TILES):
                c1, c2 = t * TILE_COLS, (t + 1) * TILE_COLS

                gAt = gApool.tile([128, T], FP16, tag="gA")
                gAv = gAt[:].rearrange("p (c f) -> p c f", f=H)
                # sweep-layer parts
                lo = c1
                while lo < min(c2, SWEEP_COLS):
                    lyr = lo // LCOLS
                    hi = min(c2, (lyr + 1) * LCOLS, SWEEP_COLS)
                    nc.sync.dma_start(
                        gAv[:, lo - c1 : hi - c1, :],
                        a_slot[:, lo - lyr * LCOLS : hi - lyr * LCOLS, :],
                    )
                    lo = hi
                # singles part
                s1 = max(c1, SINGLE_BASE)
                s2 = min(c2, SINGLE_BASE + SINGLE_COLS)
                if s1 < s2:
                    n = (s2 - s1) * 128
                    o = (s1 - SINGLE_BASE) * 8
                    nc.gpsimd.dma_gather(
                        gAv[:, s1 - c1 : s2 - c1, :],
                        a_tbl,
                        idxA_sb[:, o : o + n // 16],
                        n,
                        n,
                        H,
                        transpose=False,
                        single_packet=False,
                        queue_num=qn % 4,
                    )
                    qn += 1

                gBt = gBpool.tile([128, T], FP16, tag="gB")
                nc.gpsimd.dma_gather(
                    gBt[:].rearrange("p (c f) -> p c f", f=H),
                    b_tbl,
                    idxB_sb[:, c1 * 8 : c2 * 8],
                    T,
                    T,
                    H,
                    transpose=False,
                    single_packet=False,
                    queue_num=qn % 4,
                )
                qn += 1

                h = hpool.tile([128, T], FP16, tag="h")
                nc.vector.tensor_tensor(h[:], gAt[:], gBt[:], add_op)
                nc.scalar.activation(h[:], h[:], relu_fn)
                nc.vector.tensor_tensor(h[:], h[:], w2_rep[:], mult_op)
                red = spool.tile([128, TILE_COLS], F32, tag="red")
                nc.vector.tensor_reduce(
                    red[:],
                    h[:].rearrange("p (g f) -> p g f", f=H),
                    mybir.AxisListType.X,
                    add_op,
                )
                stage = spool.tile([128, TILE_COLS], F32, tag="stage")
                nc.scalar.activation(stage[:], red[:], ident, bias=b2_rep[:])
                nc.sync.dma_start(out[:, c1:c2], stage[:])

    nc.compile()
    return nc


# ---------------------------------------------------------------------------
# Host-side wrapper
# ---------------------------------------------------------------------------

_CACHE: dict = {}


def _get_program():
    if "nc" not in _CACHE:
        _CACHE["nc"] = _build()
    return _CACHE["nc"]


def _wrap16(flat: np.ndarray) -> np.ndarray:
    """int16 [16, n//16] with element i at [i % 16, i // 16]."""
    n = flat.shape[0]
    return np.ascontiguousarray(flat.reshape(n // 16, 16).T)


def kernel(
    x_ncRNA: np.ndarray,
    x_Protein: np.ndarray,
    edge_label_index: np.ndarray,
    W1: np.ndarray,
    b1: np.ndarray,
    W2: np.ndarray,
    b2: np.ndarray,
    _trace: bool = False,
) -> np.ndarray:
    E = edge_label_index.shape[1]
    n_nodes = x_ncRNA.shape[0]
    assert E == E_TOTAL and n_nodes == N_NODES

    i0 = np.asarray(edge_label_index[0]).astype(np.int64)
    i1 = np.asarray(edge_label_index[1]).astype(np.int64)

    nc = _get_program()

    # shared weight prep
    x_pr_t = np.zeros((H, NP), ml_dtypes.bfloat16)
    x_pr_t[:, :n_nodes] = x_Protein.T.astype(ml_dtypes.bfloat16)
    w1nc = np.ascontiguousarray(W1[:H].astype(ml_dtypes.bfloat16))
    w1pr = np.ascontiguousarray(W1[H:].astype(ml_dtypes.bfloat16))
    b1r = np.ascontiguousarray(b1.astype(ml_dtypes.bfloat16).reshape(1, H))
    w2r = np.ascontiguousarray(
        np.tile(W2[:, 0].astype(np.float16), T // H).reshape(1, T)
    )
    b2_ = np.ascontiguousarray(b2.reshape(1, 1).astype(np.float32))

    order = np.argsort(i0, kind="stable")

    in_maps = []
    slot_p = np.empty(E, np.int64)    # per (global-sorted) edge: slot partition
    slot_c = np.empty(E, np.int64)    # per edge: slot col
    for c in range(N_CORES):
        sel = order[c * E_CORE : (c + 1) * E_CORE]
        vals = i0[sel]
        v_lo = int(vals[0])
        width = int(vals[-1]) - v_lo + 1
        assert width <= WIDTH, f"core {c}: slice width {width} > {WIDTH}"
        vloc = vals - v_lo
        first = np.searchsorted(vals, vals, side="left")
        occ = np.arange(E_CORE) - first

        swept = occ < K_LAYERS
        n_single = int((~swept).sum())
        assert n_single <= SINGLE_COLS * 128, f"core {c}: {n_single} singles"

        p_arr = np.empty(E_CORE, np.int64)
        col_arr = np.empty(E_CORE, np.int64)
        p_arr[swept] = vloc[swept] % 128
        col_arr[swept] = occ[swept] * LCOLS + vloc[swept] // 128
        ks = np.arange(n_single)
        p_arr[~swept] = ks % 128
        col_arr[~swept] = SINGLE_BASE + ks // 128
        slot_p[c * E_CORE : (c + 1) * E_CORE] = p_arr
        slot_c[c * E_CORE : (c + 1) * E_CORE] = col_arr

        idxB_slot = np.zeros((128, COLS), np.int16)
        idxB_slot[p_arr, col_arr] = i1[sel].astype(np.int16)
        flatB = idxB_slot.T.reshape(-1).copy()
        flatB[SINGLE_BASE * 128 + n_single :] = -1

        flatA = np.full(SINGLE_COLS * 128, -1, np.int16)
        flatA[:n_single] = vloc[~swept].astype(np.int16)

        x_nc_t = np.zeros((H, WIDTH), ml_dtypes.bfloat16)
        x_nc_t[:, :width] = x_ncRNA[v_lo : v_lo + width].T.astype(
            ml_dtypes.bfloat16
        )

        in_maps.append(
            {
                "xt_pr": x_pr_t,
                "xt_nc": np.ascontiguousarray(x_nc_t),
                "w1nc": w1nc,
                "w1pr": w1pr,
                "b1r": b1r,
                "w2r": w2r,
                "b2": b2_,
                "idxB": _wrap16(flatB),
                "idxA": _wrap16(flatA),
            }
        )

    res = bass_utils.run_bass_kernel_spmd(
        nc, in_maps, core_ids=list(range(N_CORES)), trace=_trace
    )

    out = np.empty(E, np.float32)
    for c in range(N_CORES):
        sel = order[c * E_CORE : (c + 1) * E_CORE]
        grid = res.results[c]["out"]  # [128, COLS]
        out[sel] = grid[
            slot_p[c * E_CORE : (c + 1) * E_CORE],
            slot_c[c * E_CORE : (c + 1) * E_CORE],
        ]
    kernel._last_results = res
    return out


# revision 4
# speedup vs baseline: 1.4389x; 1.2401x over previous
"""Trainium2 Bass kernel for nn_Classifier (GNN edge-MLP link predictor).

Computes, for E candidate edges:
    out[e] = W2 . relu( x_nc[i0[e]] @ W1[:H] + x_pr[i1[e]] @ W1[H:] + b1 ) + b2

v4 strategy ("swept-A + gathered-B", 8 cores):
  The v1 bottleneck was SWDGE descriptor generation on the GpSimd Q7
  cluster: ~2.6 ns/gathered-row aggregate across all 4 queues (hard
  saturation), i.e. ~640us for 250k rows/core.  v4 removes the a-side
  descriptors:

  * Edges are range-sharded by i0: core k owns the ~125k edges whose
    i0 falls in its contiguous ~2500-row slice of the ncRNA table
    (multiplicity ~50 edges/node within the core).
  * Side A (i0): "sweep layers".  Host assigns each edge an occurrence
    rank (occ) within its i0 value; edges with occ < K=46 go to the
    slot whose (partition, col-within-layer) is derived from the
    node's permuted table row.  Layer occ is then filled by one plain
    HWDGE DMA of the whole a-slice, CONTIGUOUS per partition (row =
    p*20 + c, 5KB/partition descriptors) - zero per-edge descriptors.
    Leftover duplicates (occ >= K, <=13.3k) fill slot cols [0, 104)
    and use two dma_gathers, emitted FIRST so they run while b_tbl
    builds.  All index padding uses 0 (a valid row): trailing -1
    trimming desyncs the SWDGE ring bookkeeping (decode reserves ring
    space from the untrimmed register count) and crashes the device.
  * Side B (i1): per-tile dma_gather from the full b-table into the
    edge slots dictated by side A (131k descriptors vs 250k in v1).
  * Tables are stored row-PERMUTED: node n -> row phi(n) =
    (n//512)*512 + (n%128)*4 + (n%512)//128, which makes the phase-1
    table writes contiguous per partition (1KB descriptors instead of
    256B).  The host simply maps gather indices through phi.
  * Epilogue per tile [128 x 8192 slots]: DVE add, Scalar relu
    (offloaded from DVE), DVE mult by replicated w2, DVE free-axis
    reduce, Scalar +b2, DMA out.
  * Phase 1 builds the small a-slice first (unblocks A-singles+sweeps
    at ~15us; bias seeds live here), then the full b-table with pure
    matmuls.

Output lands as [128, COLS] f32 slot grid per core; the host scatters
slots back to original edge positions.
"""

import numpy as np
import ml_dtypes

import concourse.bass as bass
import concourse.tile as tile
from concourse import bacc, mybir
from concourse import bass_utils

F32 = mybir.dt.float32
FP16 = mybir.dt.float16
BF16 = mybir.dt.bfloat16
I16 = mybir.dt.int16

N_CORES = 8
H = 128

E_TOTAL = 1_000_000
N_NODES = 20_000
NP = 20_480         # b table rows (full, padded to 40 chunks of 512)
WIDTH = 2_560       # a table slice rows (padded; 5 chunks of 512)
K_LAYERS = 46       # sweep layers
LCOLS = WIDTH // 128            # 20 cols per layer
SINGLE_BASE = 0
SINGLE_COLS = 104               # singles capacity 13312
SWEEP_BASE = SINGLE_COLS        # 104
TILE_COLS = 64
T = TILE_COLS * 128             # 8192 slots per tile
COLS = SWEEP_BASE + K_LAYERS * LCOLS  # 1024 = 16 tiles
N_TILES = COLS // TILE_COLS
E_CORE = E_TOTAL // N_CORES

add_op = mybir.AluOpType.add
mult_op = mybir.AluOpType.mult
ident = mybir.ActivationFunctionType.Identity
relu_fn = mybir.ActivationFunctionType.Relu


def _phi(n):
    """Table-row permutation making phase-1 writes contiguous."""
    return (n // 512) * 512 + (n % 128) * 4 + (n % 512) // 128


def _build():
    nc = bacc.Bacc(
        "TRN2",
        target_bir_lowering=False,
        debug=False,
        num_devices=N_CORES,
        num_swdge_queues=4,
    )

    xt_pr = nc.dram_tensor("xt_pr", [H, NP], BF16, kind="ExternalInput").ap()
    xt_nc = nc.dram_tensor("xt_nc", [H, WIDTH], BF16, kind="ExternalInput").ap()
    w1nc = nc.dram_tensor("w1nc", [H, H], BF16, kind="ExternalInput").ap()
    w1pr = nc.dram_tensor("w1pr", [H, H], BF16, kind="ExternalInput").ap()
    b1r = nc.dram_tensor("b1r", [1, H], BF16, kind="ExternalInput").ap()
    w2r = nc.dram_tensor("w2r", [1, T], FP16, kind="ExternalInput").ap()
    b2 = nc.dram_tensor("b2", [1, 1], F32, kind="ExternalInput").ap()
    idxB = nc.dram_tensor("idxB", [16, COLS * 8], I16, kind="ExternalInput").ap()
    idxA = nc.dram_tensor("idxA", [16, SINGLE_COLS * 8], I16, kind="ExternalInput").ap()
    out = nc.dram_tensor("out", [128, COLS], F32, kind="ExternalOutput").ap()

    a_tbl = nc.dram_tensor("a_tbl", [WIDTH, H], FP16, kind="Internal").ap()
    b_tbl = nc.dram_tensor("b_tbl", [NP, H], FP16, kind="Internal").ap()

    with tile.TileContext(nc) as tc:
        with (
            tc.tile_pool(name="const", bufs=1) as cpool,
            tc.tile_pool(name="idx", bufs=1) as ipool,
            tc.tile_pool(name="x", bufs=3) as xpool,
            tc.tile_pool(name="ao", bufs=3) as apool,
            tc.tile_pool(name="gA", bufs=3) as gApool,
            tc.tile_pool(name="gB", bufs=3) as gBpool,
            tc.tile_pool(name="h", bufs=2) as hpool,
            tc.tile_pool(name="stage", bufs=2) as spool,
            tc.tile_pool(name="ps", bufs=4, space="PSUM") as pspool,
        ):
            # ---- constants ----
            w1nc_sb = cpool.tile([H, H], BF16, tag="w1nc")
            nc.sync.dma_start(w1nc_sb[:], w1nc[:])
            w1pr_sb = cpool.tile([H, H], BF16, tag="w1pr")
            nc.sync.dma_start(w1pr_sb[:], w1pr[:])
            b1_row = cpool.tile([1, H], BF16, tag="b1row")
            nc.sync.dma_start(b1_row[:], b1r[:])
            ones_sb = cpool.tile([1, H], BF16, tag="ones")
            nc.vector.memset(ones_sb[:], 1.0)
            w2_row = cpool.tile([1, T], FP16, tag="w2row")
            nc.sync.dma_start(w2_row[:], w2r[:])
            b2_sb = cpool.tile([1, 1], F32, tag="b2")
            nc.sync.dma_start(b2_sb[:], b2[:])

            w2_rep = cpool.tile([128, T], FP16, tag="w2rep")
            nc.gpsimd.partition_broadcast(w2_rep[:], w2_row[:])
            b2_rep = cpool.tile([128, 1], F32, tag="b2rep")
            nc.gpsimd.partition_broadcast(b2_rep[:], b2_sb[:])

            # ---- indices (wrapped by 16, replicated to 8 groups) ----
            idxB_sb = ipool.tile([128, COLS * 8], I16, tag="idxB")
            idxA_sb = ipool.tile([128, SINGLE_COLS * 8], I16, tag="idxA")
            for k in range(8):
                nc.sync.dma_start(idxA_sb[16 * k : 16 * (k + 1), :], idxA[:])
                nc.sync.dma_start(idxB_sb[16 * k : 16 * (k + 1), :], idxB[:])

            # ---- phase 1: a slice first (with bias seeds), then b table ----
            # permuted write view: chunk c's nodes land at rows c*512 + p*4 + k
            a_view = a_tbl.rearrange("(c p k) f -> p c k f", p=128, k=4)
            for c in range(WIDTH // 512):
                sl = slice(c * 512, (c + 1) * 512)
                xc = xpool.tile([H, 512], BF16, tag="xc")
                nc.sync.dma_start(xc[:], xt_nc[:, sl])
                ps = pspool.tile([128, 512], F32, tag="ps")
                for k in range(4):
                    ks = slice(k * 128, (k + 1) * 128)
                    nc.tensor.matmul(
                        ps[:, ks], ones_sb[:], b1_row[:], start=True, stop=False
                    )
                    nc.tensor.matmul(
                        ps[:, ks], xc[:, ks], w1nc_sb[:], start=False, stop=True
                    )
                ao = apool.tile([128, 512], FP16, tag="ao")
                nc.scalar.activation(ao[:], ps[:], ident)
                nc.sync.dma_start(
                    a_view[:, c, :, :],
                    ao[:].rearrange("p (k f) -> p k f", k=4),
                )
            b_view = b_tbl.rearrange("(c p k) f -> p c k f", p=128, k=4)
            for c in range(NP // 512):
                sl = slice(c * 512, (c + 1) * 512)
                xc = xpool.tile([H, 512], BF16, tag="xc")
                nc.sync.dma_start(xc[:], xt_pr[:, sl])
                ps = pspool.tile([128, 512], F32, tag="ps")
                for k in range(4):
                    ks = slice(k * 128, (k + 1) * 128)
                    nc.tensor.matmul(
                        ps[:, ks], xc[:, ks], w1pr_sb[:], start=True, stop=True
                    )
                ao = apool.tile([128, 512], FP16, tag="ao")
                nc.scalar.activation(ao[:], ps[:], ident)
                nc.sync.dma_start(
                    b_view[:, c, :, :],
                    ao[:].rearrange("p (k f) -> p k f", k=4),
                )

            # layer view of the a slice: slot (p, c) <-> row p*LCOLS + c
            # (contiguous 20 rows = 5KB per partition per sweep)
            a_slot = a_tbl.rearrange("(p c) f -> p c f", c=LCOLS)

            # ---- phase 2: per-tile sweep / gather / MLP ----
            qn = 0
            for t in range(N_TILES):
                c1, c2 = t * TILE_COLS, (t + 1) * TILE_COLS

                gAt = gApool.tile([128, T], FP16, tag="gA")
                gAv = gAt[:].rearrange("p (c f) -> p c f", f=H)
                # singles part (cols [0, SWEEP_BASE))
                s1, s2 = c1, min(c2, SWEEP_BASE)
                if s1 < s2:
                    n = (s2 - s1) * 128
                    o = (s1 - SINGLE_BASE) * 8
                    nc.gpsimd.dma_gather(
                        gAv[:, s1 - c1 : s2 - c1, :],
                        a_tbl,
                        idxA_sb[:, o : o + n // 16],
                        n,
                        n,
                        H,
                        transpose=False,
                        single_packet=False,
                        queue_num=qn % 4,
                    )
                    qn += 1
                # sweep-layer parts
                lo = max(c1, SWEEP_BASE)
                while lo < c2:
                    lyr = (lo - SWEEP_BASE) // LCOLS
                    hi = min(c2, SWEEP_BASE + (lyr + 1) * LCOLS)
                    lb = SWEEP_BASE + lyr * LCOLS
                    nc.sync.dma_start(
                        gAv[:, lo - c1 : hi - c1, :],
                        a_slot[:, lo - lb : hi - lb, :],
                    )
                    lo = hi

                gBt = gBpool.tile([128, T], FP16, tag="gB")
                nc.gpsimd.dma_gather(
                    gBt[:].rearrange("p (c f) -> p c f", f=H),
                    b_tbl,
                    idxB_sb[:, c1 * 8 : c2 * 8],
                    T,
                    T,
                    H,
                    transpose=False,
                    single_packet=False,
                    queue_num=qn % 4,
                )
                qn += 1

                h = hpool.tile([128, T], FP16, tag="h")
                nc.vector.tensor_tensor(h[:], gAt[:], gBt[:], add_op)
                nc.scalar.activation(h[:], h[:], relu_fn)
                nc.vector.tensor_tensor(h[:], h[:], w2_rep[:], mult_op)
                red = spool.tile([128, TILE_COLS], F32, tag="red")
                nc.vector.tensor_reduce(
                    red[:],
                    h[:].rearrange("p (g f) -> p g f", f=H),
                    mybir.AxisListType.X,
                    add_op,
                )
                stage = spool.tile([128, TILE_COLS], F32, tag="stage")
                nc.scalar.activation(stage[:], red[:], ident, bias=b2_rep[:])
                nc.sync.dma_start(out[:, c1:c2], stage[:])

    nc.compile()
    return nc


# ---------------------------------------------------------------------------
# Host-side wrapper
# ---------------------------------------------------------------------------

_CACHE: dict = {}


def _get_program():
    if "nc" not in _CACHE:
        _CACHE["nc"] = _build()
    return _CACHE["nc"]


def _wrap16(flat: np.ndarray) -> np.ndarray:
    """int16 [16, n//16] with element i at [i % 16, i // 16]."""
    n = flat.shape[0]
    return np.ascontiguousarray(flat.reshape(n // 16, 16).T)


def kernel(
    x_ncRNA: np.ndarray,
    x_Protein: np.ndarray,
    edge_label_index: np.ndarray,
    W1: np.ndarray,
    b1: np.ndarray,
    W2: np.ndarray,
    b2: np.ndarray,
    _trace: bool = False,
) -> np.ndarray:
    E = edge_label_index.shape[1]
    n_nodes = x_ncRNA.shape[0]
    assert E == E_TOTAL and n_nodes == N_NODES

    i0 = np.asarray(edge_label_index[0]).astype(np.int64)
    i1 = np.asarray(edge_label_index[1]).astype(np.int64)

    nc = _get_program()

    # shared weight prep
    x_pr_t = np.zeros((H, NP), ml_dtypes.bfloat16)
    x_pr_t[:, :n_nodes] = x_Protein.T.astype(ml_dtypes.bfloat16)
    w1nc = np.ascontiguousarray(W1[:H].astype(ml_dtypes.bfloat16))
    w1pr = np.ascontiguousarray(W1[H:].astype(ml_dtypes.bfloat16))
    b1r = np.ascontiguousarray(b1.astype(ml_dtypes.bfloat16).reshape(1, H))
    w2r = np.ascontiguousarray(
        np.tile(W2[:, 0].astype(np.float16), T // H).reshape(1, T)
    )
    b2_ = np.ascontiguousarray(b2.reshape(1, 1).astype(np.float32))

    order = np.argsort(i0, kind="stable")

    in_maps = []
    slot_p = np.empty(E, np.int64)
    slot_c = np.empty(E, np.int64)
    for c in range(N_CORES):
        sel = order[c * E_CORE : (c + 1) * E_CORE]
        vals = i0[sel]
        v_lo = int(vals[0])
        width = int(vals[-1]) - v_lo + 1
        assert width <= WIDTH, f"core {c}: slice width {width} > {WIDTH}"
        vloc = vals - v_lo
        first = np.searchsorted(vals, vals, side="left")
        occ = np.arange(E_CORE) - first

        swept = occ < K_LAYERS
        n_single = int((~swept).sum())
        assert n_single <= SINGLE_COLS * 128, f"core {c}: {n_single} singles"

        # permuted a-slice row of each edge's value
        rloc = _phi(vloc)
        p_arr = np.empty(E_CORE, np.int64)
        col_arr = np.empty(E_CORE, np.int64)
        p_arr[swept] = rloc[swept] // LCOLS
        col_arr[swept] = SWEEP_BASE + occ[swept] * LCOLS + rloc[swept] % LCOLS
        ks = np.arange(n_single)
        p_arr[~swept] = ks % 128
        col_arr[~swept] = SINGLE_BASE + ks // 128
        slot_p[c * E_CORE : (c + 1) * E_CORE] = p_arr
        slot_c[c * E_CORE : (c + 1) * E_CORE] = col_arr

        idxB_slot = np.zeros((128, COLS), np.int16)
        idxB_slot[p_arr, col_arr] = _phi(i1[sel]).astype(np.int16)
        flatB = idxB_slot.T.reshape(-1)

        flatA = np.zeros(SINGLE_COLS * 128, np.int16)
        flatA[:n_single] = rloc[~swept].astype(np.int16)

        x_nc_t = np.zeros((H, WIDTH), ml_dtypes.bfloat16)
        x_nc_t[:, :width] = x_ncRNA[v_lo : v_lo + width].T.astype(
            ml_dtypes.bfloat16
        )

        in_maps.append(
            {
                "xt_pr": x_pr_t,
                "xt_nc": np.ascontiguousarray(x_nc_t),
                "w1nc": w1nc,
                "w1pr": w1pr,
                "b1r": b1r,
                "w2r": w2r,
                "b2": b2_,
                "idxB": _wrap16(flatB),
                "idxA": _wrap16(flatA),
            }
        )

    res = bass_utils.run_bass_kernel_spmd(
        nc, in_maps, core_ids=list(range(N_CORES)), trace=_trace
    )

    out = np.empty(E, np.float32)
    for c in range(N_CORES):
        grid = res.results[c]["out"]  # [128, COLS]
        out[order[c * E_CORE : (c + 1) * E_CORE]] = grid[
            slot_p[c * E_CORE : (c + 1) * E_CORE],
            slot_c[c * E_CORE : (c + 1) * E_CORE],
        ]
    kernel._last_results = res
    return out


# revision 6
# speedup vs baseline: 1.6755x; 1.1644x over previous
"""Trainium2 Bass kernel for nn_Classifier (GNN edge-MLP link predictor).

Computes, for E candidate edges:
    out[e] = W2 . relu( x_nc[i0[e]] @ W1[:H] + x_pr[i1[e]] @ W1[H:] + b1 ) + b2

v5 strategy ("resident-A + gathered-B", 8 cores):
  v1 bottleneck: SWDGE descriptor generation on the GpSimd Q7 cluster
  saturates at ~2.6 ns/gathered-row across all 4 queues (~640us for
  250k rows/core).  v5 eliminates the a-side rows entirely:

  * Edges are range-sharded by i0: core k owns the ~125k edges whose
    i0 falls in its contiguous ~2500-row slice of the ncRNA table
    (multiplicity ~50 edges/node within the core).
  * The whole a-slice (2560 x 128 fp16 = 640KB) stays RESIDENT IN
    SBUF, written directly by the phase-1 activation (no DRAM round
    trip).  Node n's row lives at partition n%128, free col block
    (n//512)*4 + (n%512)//128 - exactly where the phase-1 psum leaves
    it.
  * Side A (i0): edges with occurrence rank occ < K=46 get slot
    (p, col = 104 + occ*20 + c) matching their a-row (p, c) in SBUF.
    The epilogue's DVE add reads the a-operand STRAIGHT from the
    resident slice (per-layer slices; 1-3 extra DVE instrs/tile) -
    zero descriptors, zero copies.  Leftover duplicates (occ >= K,
    <=13.3k) fill slot cols [0, 104) via two small dma_gathers from a
    DRAM copy of the slice, emitted first so they overlap the b-table
    build.  All index padding uses 0 (a valid row): trailing -1
    trimming desyncs SWDGE ring bookkeeping and crashes the device.
  * Side B (i1): per-tile dma_gather from the full b-table into the
    edge slots dictated by side A (131k descriptors vs 250k in v1).
  * Tables are stored row-PERMUTED: node n -> row phi(n) =
    (n//512)*512 + (n%128)*4 + (n%512)//128, making phase-1 writes
    contiguous per partition (1KB descriptors).
  * T=4096 tiles with deep, separate pools per pipeline stage so 4
    SWDGE queues stay fed (the v4 limiter was buffer-rotation
    latency, not Q7 throughput).

Output lands as [128, COLS] f32 slot grid per core; the host scatters
slots back to original edge positions.
"""

import numpy as np
import ml_dtypes

import concourse.bass as bass
import concourse.tile as tile
from concourse import bacc, mybir
from concourse import bass_utils

F32 = mybir.dt.float32
FP16 = mybir.dt.float16
BF16 = mybir.dt.bfloat16
I16 = mybir.dt.int16

N_CORES = 8
H = 128

E_TOTAL = 1_000_000
N_NODES = 20_000
NP = 20_480         # b table rows (full, padded to 40 chunks of 512)
WIDTH = 2_560       # a table slice rows (padded; 5 chunks of 512)
K_LAYERS = 46       # sweep layers
LCOLS = WIDTH // 128            # 20 cols per layer
SINGLE_BASE = 0
SINGLE_COLS = 104               # singles capacity 13312
SWEEP_BASE = SINGLE_COLS        # 104
TILE_COLS = 32
T = TILE_COLS * 128             # 4096 slots per tile
COLS = SWEEP_BASE + K_LAYERS * LCOLS  # 1024 = 32 tiles
N_TILES = COLS // TILE_COLS
E_CORE = E_TOTAL // N_CORES

add_op = mybir.AluOpType.add
mult_op = mybir.AluOpType.mult
ident = mybir.ActivationFunctionType.Identity
relu_fn = mybir.ActivationFunctionType.Relu


def _phi(n):
    """Table-row permutation making phase-1 writes contiguous."""
    return (n // 512) * 512 + (n % 128) * 4 + (n % 512) // 128


def _slot_of(n):
    """Resident a-slice position of node n: (partition, col-in-layer)."""
    return n % 128, (n // 512) * 4 + (n % 512) // 128


def _build():
    nc = bacc.Bacc(
        "TRN2",
        target_bir_lowering=False,
        debug=False,
        num_devices=N_CORES,
        num_swdge_queues=4,
    )

    xt_pr = nc.dram_tensor("xt_pr", [H, NP], BF16, kind="ExternalInput").ap()
    xt_nc = nc.dram_tensor("xt_nc", [H, WIDTH], BF16, kind="ExternalInput").ap()
    w1nc = nc.dram_tensor("w1nc", [H, H], BF16, kind="ExternalInput").ap()
    w1pr = nc.dram_tensor("w1pr", [H, H], BF16, kind="ExternalInput").ap()
    b1r = nc.dram_tensor("b1r", [1, H], BF16, kind="ExternalInput").ap()
    w2r = nc.dram_tensor("w2r", [1, T], FP16, kind="ExternalInput").ap()
    b2 = nc.dram_tensor("b2", [1, 1], F32, kind="ExternalInput").ap()
    idxB = nc.dram_tensor("idxB", [16, COLS * 8], I16, kind="ExternalInput").ap()
    idxA = nc.dram_tensor("idxA", [16, SINGLE_COLS * 8], I16, kind="ExternalInput").ap()
    out = nc.dram_tensor("out", [128, COLS], F32, kind="ExternalOutput").ap()

    a_tbl = nc.dram_tensor("a_tbl", [WIDTH, H], FP16, kind="Internal").ap()
    b_tbl = nc.dram_tensor("b_tbl", [NP, H], FP16, kind="Internal").ap()

    with tile.TileContext(nc) as tc:
        with (
            tc.tile_pool(name="const", bufs=1) as cpool,
            tc.tile_pool(name="idx", bufs=1) as ipool,
            tc.tile_pool(name="x", bufs=6) as xpool,
            tc.tile_pool(name="ao", bufs=4) as apool,
            tc.tile_pool(name="gA", bufs=2) as gApool,
            tc.tile_pool(name="gB", bufs=6) as gBpool,
            tc.tile_pool(name="h", bufs=3) as hpool,
            tc.tile_pool(name="hr", bufs=2) as rpool,
            tc.tile_pool(name="hm", bufs=2) as mpool,
            tc.tile_pool(name="stage", bufs=3) as spool,
            tc.tile_pool(name="ps", bufs=6, space="PSUM") as pspool,
        ):
            # ---- constants ----
            w1nc_sb = cpool.tile([H, H], BF16, tag="w1nc")
            nc.sync.dma_start(w1nc_sb[:], w1nc[:])
            w1pr_sb = cpool.tile([H, H], BF16, tag="w1pr")
            nc.sync.dma_start(w1pr_sb[:], w1pr[:])
            b1_row = cpool.tile([1, H], BF16, tag="b1row")
            nc.sync.dma_start(b1_row[:], b1r[:])
            ones_sb = cpool.tile([1, H], BF16, tag="ones")
            nc.vector.memset(ones_sb[:], 1.0)
            w2_row = cpool.tile([1, T], FP16, tag="w2row")
            nc.sync.dma_start(w2_row[:], w2r[:])
            b2_sb = cpool.tile([1, 1], F32, tag="b2")
            nc.sync.dma_start(b2_sb[:], b2[:])

            w2_rep = cpool.tile([128, T], FP16, tag="w2rep")
            nc.gpsimd.partition_broadcast(w2_rep[:], w2_row[:])
            b2_rep = cpool.tile([128, 1], F32, tag="b2rep")
            nc.gpsimd.partition_broadcast(b2_rep[:], b2_sb[:])

            # resident a-slice [128, WIDTH] fp16 (5KB/partition)
            a_sb = cpool.tile([128, WIDTH], FP16, tag="a_sb")

            # ---- indices (wrapped by 16, replicated to 8 groups) ----
            idxB_sb = ipool.tile([128, COLS * 8], I16, tag="idxB")
            idxA_sb = ipool.tile([128, SINGLE_COLS * 8], I16, tag="idxA")
            for k in range(8):
                nc.sync.dma_start(idxA_sb[16 * k : 16 * (k + 1), :], idxA[:])
                nc.sync.dma_start(idxB_sb[16 * k : 16 * (k + 1), :], idxB[:])

            # ---- phase 1: a slice first (with bias seeds), then b table ----
            a_view = a_tbl.rearrange("(c p k) f -> p c k f", p=128, k=4)
            for c in range(WIDTH // 512):
                sl = slice(c * 512, (c + 1) * 512)
                xc = xpool.tile([H, 512], BF16, tag="xc")
                nc.sync.dma_start(xc[:], xt_nc[:, sl])
                ps = pspool.tile([128, 512], F32, tag="ps")
                for k in range(4):
                    ks = slice(k * 128, (k + 1) * 128)
                    nc.tensor.matmul(
                        ps[:, ks], ones_sb[:], b1_row[:], start=True, stop=False
                    )
                    nc.tensor.matmul(
                        ps[:, ks], xc[:, ks], w1nc_sb[:], start=False, stop=True
                    )
                # straight into the resident slice (node c*512+k*128+p ->
                # partition p, cols (c*4+k)*128 + f == contiguous c*512 block)
                nc.scalar.activation(a_sb[:, sl], ps[:], ident)
                # DRAM copy for the singles gather
                nc.sync.dma_start(
                    a_view[:, c, :, :],
                    a_sb[:, sl].rearrange("p (k f) -> p k f", k=4),
                )
            b_view = b_tbl.rearrange("(c p k) f -> p c k f", p=128, k=4)
            for c in range(NP // 512):
                sl = slice(c * 512, (c + 1) * 512)
                xc = xpool.tile([H, 512], BF16, tag="xc")
                nc.sync.dma_start(xc[:], xt_pr[:, sl])
                ps = pspool.tile([128, 512], F32, tag="ps")
                for k in range(4):
                    ks = slice(k * 128, (k + 1) * 128)
                    nc.tensor.matmul(
                        ps[:, ks], xc[:, ks], w1pr_sb[:], start=True, stop=True
                    )
                ao = apool.tile([128, 512], FP16, tag="ao")
                nc.scalar.activation(ao[:], ps[:], ident)
                nc.sync.dma_start(
                    b_view[:, c, :, :],
                    ao[:].rearrange("p (k f) -> p k f", k=4),
                )

            # ---- phase 2: per-tile gather / MLP ----
            qn = 0
            for t in range(N_TILES):
                c1, c2 = t * TILE_COLS, (t + 1) * TILE_COLS

                # singles part (cols [0, SWEEP_BASE)) -> gA tile
                gAt = None
                s1, s2 = c1, min(c2, SWEEP_BASE)
                if s1 < s2:
                    gAt = gApool.tile([128, T], FP16, tag="gA")
                    n = (s2 - s1) * 128
                    o = (s1 - SINGLE_BASE) * 8
                    nc.gpsimd.dma_gather(
                        gAt[:, : (s2 - s1) * H].rearrange(
                            "p (c f) -> p c f", f=H
                        ),
                        a_tbl,
                        idxA_sb[:, o : o + n // 16],
                        n,
                        n,
                        H,
                        transpose=False,
                        single_packet=False,
                        queue_num=qn % 4,
                    )
                    qn += 1

                gBt = gBpool.tile([128, T], FP16, tag="gB")
                nc.gpsimd.dma_gather(
                    gBt[:].rearrange("p (c f) -> p c f", f=H),
                    b_tbl,
                    idxB_sb[:, c1 * 8 : c2 * 8],
                    T,
                    T,
                    H,
                    transpose=False,
                    single_packet=False,
                    queue_num=qn % 4,
                )
                qn += 1

                # add: a-operand from gA tile (singles) and/or resident slice
                h = hpool.tile([128, T], FP16, tag="h")
                if s1 < s2:
                    w = (s2 - s1) * H
                    nc.vector.tensor_tensor(
                        h[:, :w], gAt[:, :w], gBt[:, :w], add_op
                    )
                lo = max(c1, SWEEP_BASE)
                while lo < c2:
                    lyr = (lo - SWEEP_BASE) // LCOLS
                    hi = min(c2, SWEEP_BASE + (lyr + 1) * LCOLS)
                    lb = SWEEP_BASE + lyr * LCOLS
                    d1, d2 = (lo - c1) * H, (hi - c1) * H
                    a1, a2 = (lo - lb) * H, (hi - lb) * H
                    nc.vector.tensor_tensor(
                        h[:, d1:d2], a_sb[:, a1:a2], gBt[:, d1:d2], add_op
                    )
                    lo = hi

                hr = rpool.tile([128, T], FP16, tag="hr")
                nc.scalar.activation(hr[:], h[:], relu_fn)
                hm = mpool.tile([128, T], FP16, tag="hm")
                nc.vector.tensor_tensor(hm[:], hr[:], w2_rep[:], mult_op)
                red = spool.tile([128, TILE_COLS], F32, tag="red")
                nc.vector.tensor_reduce(
                    red[:],
                    hm[:].rearrange("p (g f) -> p g f", f=H),
                    mybir.AxisListType.X,
                    add_op,
                )
                stage = spool.tile([128, TILE_COLS], F32, tag="stage")
                nc.scalar.activation(stage[:], red[:], ident, bias=b2_rep[:])
                nc.sync.dma_start(out[:, c1:c2], stage[:])

    nc.compile()
    return nc


# ---------------------------------------------------------------------------
# Host-side wrapper
# ---------------------------------------------------------------------------

_CACHE: dict = {}


def _get_program():
    if "nc" not in _CACHE:
        _CACHE["nc"] = _build()
    return _CACHE["nc"]


def _wrap16(flat: np.ndarray) -> np.ndarray:
    """int16 [16, n//16] with element i at [i % 16, i // 16]."""
    n = flat.shape[0]
    return np.ascontiguousarray(flat.reshape(n // 16, 16).T)


def kernel(
    x_ncRNA: np.ndarray,
    x_Protein: np.ndarray,
    edge_label_index: np.ndarray,
    W1: np.ndarray,
    b1: np.ndarray,
    W2: np.ndarray,
    b2: np.ndarray,
    _trace: bool = False,
) -> np.ndarray:
    E = edge_label_index.shape[1]
    n_nodes = x_ncRNA.shape[0]
    assert E == E_TOTAL and n_nodes == N_NODES

    i0 = np.asarray(edge_label_index[0]).astype(np.int64)
    i1 = np.asarray(edge_label_index[1]).astype(np.int64)

    nc = _get_program()

    # shared weight prep
    x_pr_t = np.zeros((H, NP), ml_dtypes.bfloat16)
    x_pr_t[:, :n_nodes] = x_Protein.T.astype(ml_dtypes.bfloat16)
    w1nc = np.ascontiguousarray(W1[:H].astype(ml_dtypes.bfloat16))
    w1pr = np.ascontiguousarray(W1[H:].astype(ml_dtypes.bfloat16))
    b1r = np.ascontiguousarray(b1.astype(ml_dtypes.bfloat16).reshape(1, H))
    w2r = np.ascontiguousarray(
        np.tile(W2[:, 0].astype(np.float16), T // H).reshape(1, T)
    )
    b2_ = np.ascontiguousarray(b2.reshape(1, 1).astype(np.float32))

    order = np.argsort(i0, kind="stable")

    in_maps = []
    slot_p = np.empty(E, np.int64)
    slot_c = np.empty(E, np.int64)
    for c in range(N_CORES):
        sel = order[c * E_CORE : (c + 1) * E_CORE]
        vals = i0[sel]
        v_lo = int(vals[0])
        width = int(vals[-1]) - v_lo + 1
        assert width <= WIDTH, f"core {c}: slice width {width} > {WIDTH}"
        vloc = vals - v_lo
        first = np.searchsorted(vals, vals, side="left")
        occ = np.arange(E_CORE) - first

        swept = occ < K_LAYERS
        n_single = int((~swept).sum())
        assert n_single <= SINGLE_COLS * 128, f"core {c}: {n_single} singles"

        sp, sc = _slot_of(vloc)
        p_arr = np.empty(E_CORE, np.int64)
        col_arr = np.empty(E_CORE, np.int64)
        p_arr[swept] = sp[swept]
        col_arr[swept] = SWEEP_BASE + occ[swept] * LCOLS + sc[swept]
        ks = np.arange(n_single)
        p_arr[~swept] = ks % 128
        col_arr[~swept] = SINGLE_BASE + ks // 128
        slot_p[c * E_CORE : (c + 1) * E_CORE] = p_arr
        slot_c[c * E_CORE : (c + 1) * E_CORE] = col_arr

        idxB_slot = np.zeros((128, COLS), np.int16)
        idxB_slot[p_arr, col_arr] = _phi(i1[sel]).astype(np.int16)
        flatB = idxB_slot.T.reshape(-1)

        flatA = np.zeros(SINGLE_COLS * 128, np.int16)
        flatA[:n_single] = _phi(vloc[~swept]).astype(np.int16)

        x_nc_t = np.zeros((H, WIDTH), ml_dtypes.bfloat16)
        x_nc_t[:, :width] = x_ncRNA[v_lo : v_lo + width].T.astype(
            ml_dtypes.bfloat16
        )

        in_maps.append(
            {
                "xt_pr": x_pr_t,
                "xt_nc": np.ascontiguousarray(x_nc_t),
                "w1nc": w1nc,
                "w1pr": w1pr,
                "b1r": b1r,
                "w2r": w2r,
                "b2": b2_,
                "idxB": _wrap16(flatB),
                "idxA": _wrap16(flatA),
            }
        )

    res = bass_utils.run_bass_kernel_spmd(
        nc, in_maps, core_ids=list(range(N_CORES)), trace=_trace
    )

    out = np.empty(E, np.float32)
    for c in range(N_CORES):
        grid = res.results[c]["out"]  # [128, COLS]
        out[order[c * E_CORE : (c + 1) * E_CORE]] = grid[
            slot_p[c * E_CORE : (c + 1) * E_CORE],
            slot_c[c * E_CORE : (c + 1) * E_CORE],
        ]
    kernel._last_results = res
    return out


# revision 10
# speedup vs baseline: 1.7409x; 1.0390x over previous
"""Trainium2 Bass kernel for nn_Classifier (GNN edge-MLP link predictor).

Computes, for E candidate edges:
    out[e] = W2 . relu( x_nc[i0[e]] @ W1[:H] + x_pr[i1[e]] @ W1[H:] + b1 ) + b2

v5 strategy ("resident-A + gathered-B", 8 cores):
  v1 bottleneck: SWDGE descriptor generation on the GpSimd Q7 cluster
  saturates at ~2.6 ns/gathered-row across all 4 queues (~640us for
  250k rows/core).  v5 eliminates the a-side rows entirely:

  * Edges are range-sharded by i0: core k owns the ~125k edges whose
    i0 falls in its contiguous ~2500-row slice of the ncRNA table
    (multiplicity ~50 edges/node within the core).
  * The whole a-slice (2560 x 128 fp16 = 640KB) stays RESIDENT IN
    SBUF, written directly by the phase-1 activation (no DRAM round
    trip).  Node n's row lives at partition n%128, free col block
    (n//512)*4 + (n%512)//128 - exactly where the phase-1 psum leaves
    it.
  * Side A (i0): edges with occurrence rank occ < K=46 get slot
    (p, col = 104 + occ*20 + c) matching their a-row (p, c) in SBUF.
    The epilogue's DVE add reads the a-operand STRAIGHT from the
    resident slice (per-layer slices; 1-3 extra DVE instrs/tile) -
    zero descriptors, zero copies.  Leftover duplicates (occ >= K,
    <=13.3k) fill slot cols [0, 104) via two small dma_gathers from a
    DRAM copy of the slice, emitted first so they overlap the b-table
    build.  All index padding uses 0 (a valid row): trailing -1
    trimming desyncs SWDGE ring bookkeeping and crashes the device.
  * Side B (i1): per-tile dma_gather from the full b-table into the
    edge slots dictated by side A (131k descriptors vs 250k in v1).
  * Tables are stored row-PERMUTED: node n -> row phi(n) =
    (n//512)*512 + (n%128)*4 + (n%512)//128, making phase-1 writes
    contiguous per partition (1KB descriptors).
  * T=4096 tiles with deep, separate pools per pipeline stage so 4
    SWDGE queues stay fed (the v4 limiter was buffer-rotation
    latency, not Q7 throughput).

Output lands as [128, COLS] f32 slot grid per core; the host scatters
slots back to original edge positions.
"""

import numpy as np
import ml_dtypes

import concourse.bass as bass
import concourse.tile as tile
from concourse import bacc, mybir
from concourse import bass_utils

F32 = mybir.dt.float32
FP16 = mybir.dt.float16
BF16 = mybir.dt.bfloat16
I16 = mybir.dt.int16

N_CORES = 8
H = 128

E_TOTAL = 1_000_000
N_NODES = 20_000
NP = 20_480         # b table rows (full, padded to 40 chunks of 512)
WIDTH = 2_560       # a table slice rows (padded; 5 chunks of 512)
K_LAYERS = 46       # sweep layers
LCOLS = WIDTH // 128            # 20 cols per layer
SINGLE_BASE = 0
SINGLE_COLS = 104               # singles capacity 13312
SWEEP_BASE = SINGLE_COLS        # 104
TILE_COLS = 32
T = TILE_COLS * 128             # 4096 slots per tile
COLS = SWEEP_BASE + K_LAYERS * LCOLS  # 1024 = 32 tiles
N_TILES = COLS // TILE_COLS
E_CORE = E_TOTAL // N_CORES

add_op = mybir.AluOpType.add
mult_op = mybir.AluOpType.mult
ident = mybir.ActivationFunctionType.Identity
relu_fn = mybir.ActivationFunctionType.Relu


def _phi(n):
    """Table-row permutation making phase-1 writes contiguous."""
    return (n // 512) * 512 + (n % 128) * 4 + (n % 512) // 128


def _slot_of(n):
    """Resident a-slice position of node n: (partition, col-in-layer)."""
    return n % 128, (n // 512) * 4 + (n % 512) // 128


def _build():
    nc = bacc.Bacc(
        "TRN2",
        target_bir_lowering=False,
        debug=False,
        num_devices=N_CORES,
        num_swdge_queues=4,
    )

    xt_pr = nc.dram_tensor("xt_pr", [H, NP], BF16, kind="ExternalInput").ap()
    xt_nc = nc.dram_tensor("xt_nc", [H, WIDTH], BF16, kind="ExternalInput").ap()
    w1nc = nc.dram_tensor("w1nc", [H, H], BF16, kind="ExternalInput").ap()
    w1pr = nc.dram_tensor("w1pr", [H, H], BF16, kind="ExternalInput").ap()
    b1r = nc.dram_tensor("b1r", [1, H], BF16, kind="ExternalInput").ap()
    w2r = nc.dram_tensor("w2r", [1, T], FP16, kind="ExternalInput").ap()
    b2 = nc.dram_tensor("b2", [1, 1], F32, kind="ExternalInput").ap()
    idxB = nc.dram_tensor("idxB", [16, COLS * 8], I16, kind="ExternalInput").ap()
    idxA = nc.dram_tensor("idxA", [16, SINGLE_COLS * 8], I16, kind="ExternalInput").ap()
    out = nc.dram_tensor("out", [128, COLS], F32, kind="ExternalOutput").ap()

    a_tbl = nc.dram_tensor("a_tbl", [WIDTH, H], FP16, kind="Internal").ap()
    b_tbl = nc.dram_tensor("b_tbl", [NP, H], FP16, kind="Internal").ap()

    with tile.TileContext(nc) as tc:
        with (
            tc.tile_pool(name="const", bufs=1) as cpool,
            tc.tile_pool(name="idx", bufs=1) as ipool,
            tc.tile_pool(name="x", bufs=6) as xpool,
            tc.tile_pool(name="ao", bufs=4) as apool,
            tc.tile_pool(name="gA", bufs=2) as gApool,
            tc.tile_pool(name="gB", bufs=7) as gBpool,
            tc.tile_pool(name="h", bufs=3) as hpool,
            tc.tile_pool(name="hr", bufs=2) as rpool,
            tc.tile_pool(name="hm", bufs=2) as mpool,
            tc.tile_pool(name="stage", bufs=3) as spool,
            tc.tile_pool(name="ps", bufs=6, space="PSUM") as pspool,
        ):
            # ---- constants ----
            w1nc_sb = cpool.tile([H, H], BF16, tag="w1nc")
            nc.sync.dma_start(w1nc_sb[:], w1nc[:])
            w1pr_sb = cpool.tile([H, H], BF16, tag="w1pr")
            nc.sync.dma_start(w1pr_sb[:], w1pr[:])
            b1_row = cpool.tile([1, H], BF16, tag="b1row")
            nc.sync.dma_start(b1_row[:], b1r[:])
            ones_sb = cpool.tile([1, H], BF16, tag="ones")
            nc.vector.memset(ones_sb[:], 1.0)
            w2_row = cpool.tile([1, T], FP16, tag="w2row")
            nc.sync.dma_start(w2_row[:], w2r[:])
            b2_sb = cpool.tile([1, 1], F32, tag="b2")
            nc.sync.dma_start(b2_sb[:], b2[:])

            w2_rep = cpool.tile([128, T], FP16, tag="w2rep")
            nc.gpsimd.partition_broadcast(w2_rep[:], w2_row[:])
            b2_rep = cpool.tile([128, 1], F32, tag="b2rep")
            nc.gpsimd.partition_broadcast(b2_rep[:], b2_sb[:])

            # resident a-slice [128, WIDTH] fp16 (5KB/partition)
            a_sb = cpool.tile([128, WIDTH], FP16, tag="a_sb")

            # ---- indices (wrapped by 16, replicated to 8 groups) ----
            idxB_sb = ipool.tile([128, COLS * 8], I16, tag="idxB")
            idxA_sb = ipool.tile([128, SINGLE_COLS * 8], I16, tag="idxA")
            for k in range(8):
                nc.sync.dma_start(idxA_sb[16 * k : 16 * (k + 1), :], idxA[:])
                nc.sync.dma_start(idxB_sb[16 * k : 16 * (k + 1), :], idxB[:])

            # ---- phase 1 ----
            # order: b chunks 0-15 (unblocks the i1-sorted singles B-gathers
            # on the b_tbl prefix), then the a slice (unblocks A-singles and
            # the resident adds), then b chunks 16-39.
            a_view = a_tbl.rearrange("(c p k) f -> p c k f", p=128, k=4)
            b_view = b_tbl.rearrange("(c p k) f -> p c k f", p=128, k=4)

            def b_chunk(c):
                sl = slice(c * 512, (c + 1) * 512)
                xc = xpool.tile([H, 512], BF16, tag="xc")
                nc.sync.dma_start(xc[:], xt_pr[:, sl])
                ps = pspool.tile([128, 512], F32, tag="ps")
                for k in range(4):
                    ks = slice(k * 128, (k + 1) * 128)
                    nc.tensor.matmul(
                        ps[:, ks], xc[:, ks], w1pr_sb[:], start=True, stop=True
                    )
                ao = apool.tile([128, 512], FP16, tag="ao")
                nc.scalar.activation(ao[:], ps[:], ident)
                nc.sync.dma_start(
                    b_view[:, c, :, :],
                    ao[:].rearrange("p (k f) -> p k f", k=4),
                )

            for c in range(16):
                b_chunk(c)
            for c in range(WIDTH // 512):
                sl = slice(c * 512, (c + 1) * 512)
                xc = xpool.tile([H, 512], BF16, tag="xc")
                nc.sync.dma_start(xc[:], xt_nc[:, sl])
                ps = pspool.tile([128, 512], F32, tag="ps")
                for k in range(4):
                    ks = slice(k * 128, (k + 1) * 128)
                    nc.tensor.matmul(
                        ps[:, ks], ones_sb[:], b1_row[:], start=True, stop=False
                    )
                    nc.tensor.matmul(
                        ps[:, ks], xc[:, ks], w1nc_sb[:], start=False, stop=True
                    )
                # straight into the resident slice (node c*512+k*128+p ->
                # partition p, cols (c*4+k)*128 + f == contiguous c*512 block)
                nc.scalar.activation(a_sb[:, sl], ps[:], ident)
                # DRAM copy for the singles gather
                nc.sync.dma_start(
                    a_view[:, c, :, :],
                    a_sb[:, sl].rearrange("p (k f) -> p k f", k=4),
                )
            for c in range(16, NP // 512):
                b_chunk(c)

            # ---- phase 2: per-tile gather / MLP ----
            qn = 0
            for t in range(N_TILES):
                c1, c2 = t * TILE_COLS, (t + 1) * TILE_COLS

                # singles part (cols [0, SWEEP_BASE)) -> gA tile
                gAt = None
                s1, s2 = c1, min(c2, SWEEP_BASE)
                if s1 < s2:
                    gAt = gApool.tile([128, T], FP16, tag="gA")
                    n = (s2 - s1) * 128
                    o = (s1 - SINGLE_BASE) * 8
                    nc.gpsimd.dma_gather(
                        gAt[:, : (s2 - s1) * H].rearrange(
                            "p (c f) -> p c f", f=H
                        ),
                        a_tbl,
                        idxA_sb[:, o : o + n // 16],
                        n,
                        n,
                        H,
                        transpose=False,
                        single_packet=False,
                        queue_num=qn % 4,
                    )
                    qn += 1

                # i1-sorted singles let the first two tiles depend only on a
                # prefix of b_tbl (host asserts the index bounds hold)
                b_src = b_tbl
                if t == 0:
                    b_src = b_tbl[0:8192]
                elif t == 1:
                    b_src = b_tbl[0:15872]
                gBt = gBpool.tile([128, T], FP16, tag="gB")
                nc.gpsimd.dma_gather(
                    gBt[:].rearrange("p (c f) -> p c f", f=H),
                    b_src,
                    idxB_sb[:, c1 * 8 : c2 * 8],
                    T,
                    T,
                    H,
                    transpose=False,
                    single_packet=False,
                    queue_num=qn % 4,
                )
                qn += 1

                # add: a-operand from gA tile (singles) and/or resident slice
                h = hpool.tile([128, T], FP16, tag="h")
                if s1 < s2:
                    w = (s2 - s1) * H
                    nc.vector.tensor_tensor(
                        h[:, :w], gAt[:, :w], gBt[:, :w], add_op
                    )
                lo = max(c1, SWEEP_BASE)
                while lo < c2:
                    lyr = (lo - SWEEP_BASE) // LCOLS
                    hi = min(c2, SWEEP_BASE + (lyr + 1) * LCOLS)
                    lb = SWEEP_BASE + lyr * LCOLS
                    d1, d2 = (lo - c1) * H, (hi - c1) * H
                    a1, a2 = (lo - lb) * H, (hi - lb) * H
                    nc.vector.tensor_tensor(
                        h[:, d1:d2], a_sb[:, a1:a2], gBt[:, d1:d2], add_op
                    )
                    lo = hi

                hr = rpool.tile([128, T], FP16, tag="hr")
                nc.scalar.activation(hr[:], h[:], relu_fn)
                hm = mpool.tile([128, T], FP16, tag="hm")
                nc.vector.tensor_tensor(hm[:], hr[:], w2_rep[:], mult_op)
                red = spool.tile([128, TILE_COLS], F32, tag="red")
                nc.vector.tensor_reduce(
                    red[:],
                    hm[:].rearrange("p (g f) -> p g f", f=H),
                    mybir.AxisListType.X,
                    add_op,
                )
                stage = spool.tile([128, TILE_COLS], F32, tag="stage")
                nc.scalar.activation(stage[:], red[:], ident, bias=b2_rep[:])
                nc.sync.dma_start(out[:, c1:c2], stage[:])

    nc.compile()
    return nc


# ---------------------------------------------------------------------------
# Host-side wrapper
# ---------------------------------------------------------------------------

_CACHE: dict = {}


def _get_program():
    if "nc" not in _CACHE:
        _CACHE["nc"] = _build()
    return _CACHE["nc"]


def _wrap16(flat: np.ndarray) -> np.ndarray:
    """int16 [16, n//16] with element i at [i % 16, i // 16]."""
    n = flat.shape[0]
    return np.ascontiguousarray(flat.reshape(n // 16, 16).T)


def kernel(
    x_ncRNA: np.ndarray,
    x_Protein: np.ndarray,
    edge_label_index: np.ndarray,
    W1: np.ndarray,
    b1: np.ndarray,
    W2: np.ndarray,
    b2: np.ndarray,
    _trace: bool = False,
) -> np.ndarray:
    E = edge_label_index.shape[1]
    n_nodes = x_ncRNA.shape[0]
    assert E == E_TOTAL and n_nodes == N_NODES

    i0 = np.asarray(edge_label_index[0]).astype(np.int64)
    i1 = np.asarray(edge_label_index[1]).astype(np.int64)

    nc = _get_program()

    # shared weight prep
    x_pr_t = np.zeros((H, NP), ml_dtypes.bfloat16)
    x_pr_t[:, :n_nodes] = x_Protein.T.astype(ml_dtypes.bfloat16)
    w1nc = np.ascontiguousarray(W1[:H].astype(ml_dtypes.bfloat16))
    w1pr = np.ascontiguousarray(W1[H:].astype(ml_dtypes.bfloat16))
    b1r = np.ascontiguousarray(b1.astype(ml_dtypes.bfloat16).reshape(1, H))
    w2r = np.ascontiguousarray(
        np.tile(W2[:, 0].astype(np.float16), T // H).reshape(1, T)
    )
    b2_ = np.ascontiguousarray(b2.reshape(1, 1).astype(np.float32))

    order = np.argsort(i0, kind="stable")

    in_maps = []
    slot_p = np.empty(E, np.int64)
    slot_c = np.empty(E, np.int64)
    for c in range(N_CORES):
        sel = order[c * E_CORE : (c + 1) * E_CORE]
        vals = i0[sel]
        v_lo = int(vals[0])
        width = int(vals[-1]) - v_lo + 1
        assert width <= WIDTH, f"core {c}: slice width {width} > {WIDTH}"
        vloc = vals - v_lo
        first = np.searchsorted(vals, vals, side="left")
        occ = np.arange(E_CORE) - first

        swept = occ < K_LAYERS
        n_single = int((~swept).sum())
        assert n_single <= SINGLE_COLS * 128, f"core {c}: {n_single} singles"

        sp, sc = _slot_of(vloc)
        p_arr = np.empty(E_CORE, np.int64)
        col_arr = np.empty(E_CORE, np.int64)
        p_arr[swept] = sp[swept]
        col_arr[swept] = SWEEP_BASE + occ[swept] * LCOLS + sc[swept]
        # singles sorted by i1 so the first tiles only touch a b_tbl prefix
        sidx = np.where(~swept)[0]
        sidx = sidx[np.argsort(i1[sel][sidx], kind="stable")]
        ks = np.arange(n_single)
        p_arr[sidx] = ks % 128
        col_arr[sidx] = SINGLE_BASE + ks // 128
        s_i1 = i1[sel][sidx]
        if n_single > 4096:
            assert s_i1[4095] < 8192, f"core {c}: tile0 b-prefix violated"
        if n_single > 8192:
            assert s_i1[8191] < 15872, f"core {c}: tile1 b-prefix violated"
        slot_p[c * E_CORE : (c + 1) * E_CORE] = p_arr
        slot_c[c * E_CORE : (c + 1) * E_CORE] = col_arr

        idxB_slot = np.zeros((128, COLS), np.int16)
        idxB_slot[p_arr, col_arr] = _phi(i1[sel]).astype(np.int16)
        flatB = idxB_slot.T.reshape(-1)

        flatA = np.zeros(SINGLE_COLS * 128, np.int16)
        flatA[:n_single] = _phi(vloc[sidx]).astype(np.int16)

        x_nc_t = np.zeros((H, WIDTH), ml_dtypes.bfloat16)
        x_nc_t[:, :width] = x_ncRNA[v_lo : v_lo + width].T.astype(
            ml_dtypes.bfloat16
        )

        in_maps.append(
            {
                "xt_pr": x_pr_t,
                "xt_nc": np.ascontiguousarray(x_nc_t),
                "w1nc": w1nc,
                "w1pr": w1pr,
                "b1r": b1r,
                "w2r": w2r,
                "b2": b2_,
                "idxB": _wrap16(flatB),
                "idxA": _wrap16(flatA),
            }
        )

    res = bass_utils.run_bass_kernel_spmd(
        nc, in_maps, core_ids=list(range(N_CORES)), trace=_trace
    )

    out = np.empty(E, np.float32)
    for c in range(N_CORES):
        grid = res.results[c]["out"]  # [128, COLS]
        out[order[c * E_CORE : (c + 1) * E_CORE]] = grid[
            slot_p[c * E_CORE : (c + 1) * E_CORE],
            slot_c[c * E_CORE : (c + 1) * E_CORE],
        ]
    kernel._last_results = res
    return out
